# revision 10
# baseline (speedup 1.0000x reference)
"""TSSA causal self-attention on 8 Trainium2 NeuronCores.

Sharding: 4-way data-parallel over B x 2-way tensor-parallel over heads.
Core c handles batch b = c//2 and heads [8*(c%2), 8*(c%2)+8).

Per core:
  1) w^T = Wa_half @ x_b^T in (feat, T) layout (fp32r matmuls), spilling w to
     DRAM; chains w_sq -> clamped running-denom scan -> reciprocal ->
     per-head PE ones-reduce into per-head tmp rows (written to DRAM).
  2) Two pairwise AllGathers (one per T-half) so the head-softmax of the
     first half overlaps stage-1 compute of the second half. Softmax across
     all 16 heads (mean-shifted, exp on ACT), extraction of this core's 8
     Pi rows via a selection matmul, cumsum of Pi and -Pi*cumPi' rows.
  3) Re-reads w, rebuilds w_sq (ACT), forms den2 = cumPi' + cum(w_sq*Pi)
     with one fused scan of Pi*(w_sq+1), y = w * (-Pi*cumPi') / den2
     (broadcasts + products on GPSIMD, scan/recip on DVE), projects through
     Wp_half into a partial (T, C) output.
Host sums the two partial outputs of each batch pair.
"""
import sys

sys.path.insert(0, "/opt/trn_rl_repo")

import numpy as np

import concourse.bacc as bacc
import concourse.mybir as mybir
import concourse.tile as tile
from concourse.bass_utils import run_bass_kernel_spmd

B, T, C, H, D = 4, 4096, 2048, 16, 128
NCORES = 8
HPC = H // 2          # heads per core
F = HPC * D           # features per core (1024)
S1 = 512              # stage-1 slab width along T
S2 = 512              # stage-2 slab width along T
TH = T // 2           # collective half width
F32 = mybir.dt.float32
FR = mybir.dt.float32r
ADD = mybir.AluOpType.add
MULT = mybir.AluOpType.mult
SUB = mybir.AluOpType.subtract
MAX = mybir.AluOpType.max
BYP = mybir.AluOpType.bypass
GROUPS = [[0, 1], [2, 3], [4, 5], [6, 7]]

_cached_nc = None


def _build():
    nc = bacc.Bacc("TRN2", target_bir_lowering=False, debug=False,
                   num_devices=NCORES)

    xT_d = nc.dram_tensor("xT", [C, T], FR, kind="ExternalInput").ap()
    wa_d = nc.dram_tensor("wa", [C, F], FR, kind="ExternalInput").ap()
    wp_d = nc.dram_tensor("wp", [F, C], FR, kind="ExternalInput").ap()
    bt_d = nc.dram_tensor("btall", [H, T], F32, kind="ExternalInput").ap()
    tl_d = nc.dram_tensor("tmplall", [H, 1], F32, kind="ExternalInput").ap()
    sel_d = nc.dram_tensor("sel", [H, HPC], FR, kind="ExternalInput").ap()
    ones_d = nc.dram_tensor("ones", [D, 1], FR, kind="ExternalInput").ap()
    o16m_d = nc.dram_tensor("o16m", [H, H], FR, kind="ExternalInput").ap()
    o16s_d = nc.dram_tensor("o16s", [H, H], FR, kind="ExternalInput").ap()
    out_d = nc.dram_tensor("out_p", [T, C], F32, kind="ExternalOutput").ap()

    with tile.TileContext(nc) as tc:
        with tc.tile_pool(name="dram", bufs=1, space="DRAM") as dp, \
             tc.tile_pool(name="const", bufs=1) as cp, \
             tc.tile_pool(name="sm", bufs=1) as sm, \
             tc.tile_pool(name="sm2", bufs=2) as sm2, \
             tc.tile_pool(name="smp", bufs=2, space="PSUM") as smp:
            w_spill = dp.tile([F, T], F32)
            cc_in = [dp.tile([HPC, TH], F32, name=f"cc_in{i}")
                     for i in range(2)]
            cc_out = [dp.tile([H, TH], F32, name=f"cc_out{i}")
                      for i in range(2)]

            myp = cp.tile([HPC, T], F32)
            npc = cp.tile([HPC, T], F32)
            carry_cpi = cp.tile([HPC, 1], F32)
            nc.vector.memset(carry_cpi, 1e-8)

            ones16m = sm.sbuf_tile_from(o16m_d)
            ones16s = sm.sbuf_tile_from(o16s_d)
            bt_sb = sm.sbuf_tile_from(bt_d)
            tl_sb = sm.sbuf_tile_from(tl_d)
            sel_sb = sm.sbuf_tile_from(sel_d)

            def softmax_half(hf):
                c0 = hf * TH
                nc.gpsimd.collective_compute(
                    "AllGather", BYP, replica_groups=GROUPS,
                    ins=[cc_in[hf][:, :]], outs=[cc_out[hf][:, :]])
                ch = sm.tile([H, TH], FR, tag="ch", name=f"ch{hf}")
                nc.sync.dma_start(ch.bitcast(F32), cc_out[hf])
                # tmp = gathered * temp + D*bias*temp
                nc.vector.scalar_tensor_tensor(
                    ch, ch.bitcast(F32), tl_sb, bt_sb[:, c0:c0 + TH],
                    MULT, ADD)
                for nb in range(TH // 512):
                    n0, n1 = nb * 512, (nb + 1) * 512
                    pm = smp.tile([H, 512], F32, tag="smp",
                                  name=f"pm{hf}_{nb}")
                    nc.tensor.matmul(pm, ones16m, ch[:, n0:n1],
                                     start=True, stop=True)
                    nc.vector.tensor_tensor(ch[:, n0:n1], ch[:, n0:n1],
                                            pm, SUB)
                nc.scalar.activation(ch, ch,
                                     mybir.ActivationFunctionType.Exp)
                for nb in range(TH // 512):
                    n0, n1 = nb * 512, (nb + 1) * 512
                    ps_ = smp.tile([H, 512], F32, tag="smp",
                                   name=f"ps{hf}_{nb}")
                    nc.tensor.matmul(ps_, ones16s, ch[:, n0:n1],
                                     start=True, stop=True)
                    rs = sm2.tile([H, 512], F32, tag="rs",
                                  name=f"rs{hf}_{nb}")
                    nc.vector.reciprocal_approx_fast(rs, ps_)
                    nc.vector.tensor_tensor(ch[:, n0:n1], ch[:, n0:n1],
                                            rs, MULT)
                for nb in range(TH // 512):
                    n0, n1 = nb * 512, (nb + 1) * 512
                    mp = smp.tile([HPC, 512], F32, tag="smp",
                                  name=f"mp{hf}_{nb}")
                    nc.tensor.matmul(mp, sel_sb, ch[:, n0:n1],
                                     start=True, stop=True)
                    nc.scalar.copy(myp[:, c0 + n0:c0 + n1], mp)
                cpi = sm.tile([HPC, TH], F32, tag="cpi", name=f"cpi{hf}")
                nc.vector.tensor_tensor_scan(
                    cpi, myp[:, c0:c0 + TH], myp[:, c0:c0 + TH],
                    carry_cpi, ADD, BYP)
                nc.vector.tensor_copy(carry_cpi, cpi[:, TH - 1:TH])
                nc.vector.scalar_tensor_tensor(
                    npc[:, c0:c0 + TH], myp[:, c0:c0 + TH], -1.0, cpi,
                    MULT, MULT)

            # ---------------- stage 1 ----------------
            with tc.tile_pool(name="s1w", bufs=1) as pw1, \
                 tc.tile_pool(name="s1x", bufs=17) as px, \
                 tc.tile_pool(name="s1a", bufs=2) as pa, \
                 tc.tile_pool(name="s1ps", bufs=3, space="PSUM") as ps1, \
                 tc.tile_pool(name="s1pc", bufs=2, space="PSUM") as psc:
                wa_sb = pw1.tile([D, 16 * F], FR)
                for k in range(16):
                    nc.sync.dma_start(wa_sb[:, k * F:(k + 1) * F],
                                      wa_d[k * D:(k + 1) * D, :])
                ones = pw1.sbuf_tile_from(ones_d)
                eps12 = pw1.tile([D, 1], F32)
                nc.vector.memset(eps12, 1e-12)
                carry_d = pw1.tile([D, HPC], F32)
                nc.vector.memset(carry_d, 0.0)

                for s in range(T // S1):
                    c0, c1 = s * S1, (s + 1) * S1
                    hf, hcol = s * S1 // TH, (s * S1) % TH
                    xs = []
                    for k in range(16):
                        xk = px.tile([D, S1], FR, tag="xs", name=f"xs{s}_{k}")
                        nc.sync.dma_start(xk, xT_d[k * D:(k + 1) * D, c0:c1])
                        xs.append(xk)
                    for h in range(HPC):
                        pw = ps1.tile([D, S1], F32, tag="pw",
                                      name=f"pw{s}_{h}")
                        for k in range(16):
                            nc.tensor.matmul(
                                pw,
                                wa_sb[:, k * F + h * D:k * F + (h + 1) * D],
                                xs[k], start=(k == 0), stop=(k == 15))
                        w_sb = pa.tile([D, S1], F32, tag="w_sb",
                                       name=f"wsb{s}_{h}")
                        nc.scalar.copy(w_sb, pw)
                        nc.sync.dma_start(
                            w_spill[h * D:(h + 1) * D, c0:c1], w_sb)
                        wsq = pa.tile([D, S1], F32, tag="wsq",
                                      name=f"wsq{s}_{h}")
                        nc.scalar.square(wsq, pw)
                        den = pa.tile([D, S1], F32, tag="den",
                                      name=f"den{s}_{h}")
                        nc.vector.tensor_tensor_scan(
                            den, wsq, eps12.broadcast_to((D, S1)),
                            carry_d[:, h:h + 1], ADD, MAX)
                        nc.vector.tensor_copy(carry_d[:, h:h + 1],
                                              den[:, S1 - 1:S1])
                        rden = pa.tile([D, S1], F32, tag="rden",
                                       name=f"rden{s}_{h}")
                        nc.vector.reciprocal_approx_fast(rden, den)
                        r = pa.tile([D, S1], FR, tag="r", name=f"r{s}_{h}")
                        nc.vector.tensor_tensor(r, wsq, rden, MULT)
                        pc = psc.tile([1, S1], F32, tag="pc",
                                      name=f"pc{s}_{h}")
                        nc.tensor.matmul(pc, ones, r, start=True, stop=True)
                        trow = pa.tile([1, S1], F32, tag="trow",
                                       name=f"trow{s}_{h}")
                        nc.scalar.copy(trow, pc)
                        nc.sync.dma_start(
                            cc_in[hf][h:h + 1, hcol:hcol + S1], trow)
                    if s == T // S1 // 2 - 1:
                        softmax_half(0)
                softmax_half(1)

            # ---------------- stage 2 ----------------
            with tc.tile_pool(name="s2w", bufs=1) as pw2, \
                 tc.tile_pool(name="s2a", bufs=2) as p2, \
                 tc.tile_pool(name="s2y", bufs=2) as py, \
                 tc.tile_pool(name="s2oc", bufs=3) as poc, \
                 tc.tile_pool(name="s2pt", bufs=6, space="PSUM") as pps:
                wp_sb = pw2.tile([D, HPC * C], FR)
                for h in range(HPC):
                    nc.sync.dma_start(wp_sb[:, h * C:(h + 1) * C],
                                      wp_d[h * D:(h + 1) * D, :])
                carry_2 = pw2.tile([D, HPC], F32)
                nc.vector.memset(carry_2, 1e-8)

                for s in range(T // S2):
                    c0, c1 = s * S2, (s + 1) * S2
                    ys = []
                    for h in range(HPC):
                        w2 = p2.tile([D, S2], F32, tag="w2",
                                     name=f"w2_{s}_{h}")
                        nc.sync.dma_start(
                            w2, w_spill[h * D:(h + 1) * D, c0:c1])
                        t2 = p2.tile([D, S2], F32, tag="t2",
                                     name=f"t2_{s}_{h}")
                        nc.scalar.square(t2, w2)
                        rowp = p2.tile([1, S2], F32, tag="rowp",
                                       name=f"rowp{s}_{h}")
                        nc.sync.dma_start(rowp, myp[h:h + 1, c0:c1])
                        pib = p2.tile([D, S2], F32, tag="pib",
                                      name=f"pib{s}_{h}")
                        nc.gpsimd.partition_broadcast(pib, rowp)
                        nc.vector.scalar_tensor_tensor(t2, t2, 1.0, pib,
                                                       ADD, MULT)
                        nc.vector.tensor_tensor_scan(
                            t2, t2, t2, carry_2[:, h:h + 1], ADD, BYP)
                        nc.vector.tensor_copy(carry_2[:, h:h + 1],
                                              t2[:, S2 - 1:S2])
                        nc.vector.reciprocal_approx_fast(t2, t2)
                        rown = p2.tile([1, S2], F32, tag="rown",
                                       name=f"rown{s}_{h}")
                        nc.sync.dma_start(rown, npc[h:h + 1, c0:c1])
                        npcb = p2.tile([D, S2], F32, tag="npcb",
                                       name=f"npcb{s}_{h}")
                        nc.gpsimd.partition_broadcast(npcb, rown)
                        nc.gpsimd.tensor_tensor(w2, w2, npcb, MULT)
                        y_h = py.tile([D, S2], FR, tag=f"y{h}",
                                      name=f"y{s}_{h}")
                        nc.gpsimd.tensor_tensor(y_h, w2, t2, MULT)
                        ys.append(y_h)
                    for tb in range(S2 // D):
                        for ob in range(C // 512):
                            o0, o1 = ob * 512, (ob + 1) * 512
                            pt = pps.tile([D, 512], F32, tag="pt",
                                          name=f"pt{s}_{tb}_{ob}")
                            for h in range(HPC):
                                nc.tensor.matmul(
                                    pt,
                                    ys[h][:, tb * D:(tb + 1) * D],
                                    wp_sb[:, h * C + o0:h * C + o1],
                                    start=(h == 0), stop=(h == HPC - 1))
                            oc = poc.tile([D, 512], F32, tag="oc",
                                          name=f"oc{s}_{tb}_{ob}")
                            nc.scalar.copy(oc, pt)
                            nc.sync.dma_start(
                                out_d[c0 + tb * D:c0 + (tb + 1) * D, o0:o1],
                                oc)

    nc.compile()
    return nc


def _prep_inputs(x, Wa, Wp, temp, denom_bias):
    x = np.asarray(x, dtype=np.float32)
    Wa = np.asarray(Wa, dtype=np.float32)
    Wp = np.asarray(Wp, dtype=np.float32)
    temp = np.asarray(temp, dtype=np.float32)
    denom_bias = np.asarray(denom_bias, dtype=np.float32)

    btall = (D * denom_bias[:, :T, 0] * temp).astype(np.float32)  # (H, T)
    tmplall = temp.reshape(H, 1).astype(np.float32)

    in_maps = []
    for c in range(NCORES):
        b, half = c // 2, c % 2
        fsel = slice(half * F, (half + 1) * F)
        sel = np.zeros((H, HPC), np.float32)
        for j in range(HPC):
            sel[half * HPC + j, j] = 1.0
        in_maps.append({
            "xT": np.ascontiguousarray(x[b].T),
            "wa": np.ascontiguousarray(Wa[fsel, :].T),
            "wp": np.ascontiguousarray(Wp[:, fsel].T),
            "btall": btall,
            "tmplall": tmplall,
            "sel": sel,
            "ones": np.ones((D, 1), np.float32),
            "o16m": np.full((H, H), 1.0 / H, np.float32),
            "o16s": np.ones((H, H), np.float32),
        })
    return in_maps


def _run(in_maps, trace=False, tmpdir=None):
    global _cached_nc
    if _cached_nc is None:
        _cached_nc = _build()
    return run_bass_kernel_spmd(_cached_nc, in_maps,
                                core_ids=list(range(NCORES)), trace=trace,
                                tmpdir=tmpdir)


def kernel(x, Wa, Wp, temp, denom_bias):
    in_maps = _prep_inputs(x, Wa, Wp, temp, denom_bias)
    res = _run(in_maps)
    out = np.empty((B, T, C), np.float32)
    for b in range(B):
        out[b] = res.results[2 * b]["out_p"] + res.results[2 * b + 1]["out_p"]
    return out
